# revision 1
# baseline (speedup 1.0000x reference)
"""BandSplitLinear Trainium2 kernel (v3: fp16 PE datapath, PE transposes).

Strategy (per core, batch-parallel over 8 cores):
  - No nonlinearity between the two per-band linears -> fold w_pre @ w_post
    into one 128x128 matrix per band on the host (6x fewer FLOPs). Biases are
    additive constants per (c, f) -> applied host-side.
  - Bands are disjoint contiguous frequency ranges. Carve the frequency axis
    into 33 aligned segments of 32 bins; per segment use the 128-partition
    feature layout g = c*32 + u. Every band spans <= 2 segments, so the whole
    computation becomes y.T[seg_out] = sum_{seg_in} Wg[seg_in, seg_out].T @
    x.T[seg_in] over 97 host-built zero-padded 128x128 blocks. Gather/scatter
    vanish into the weight sparsity pattern.
  - fp16 datapath on chip (fp32 PSUM accumulation): SWDGE cast-DMA loads,
    DVE pack into segment-major layout, PE transposes (1 cyc/row at fp16),
    fp16 matmuls with resident weights, PE transposes back, strided copies
    into output staging, SWDGE cast-DMA stores.
"""

import numpy as np

import concourse.bass as bass
import concourse.tile as tile
from concourse import bacc, mybir
from concourse.bass_utils import run_bass_kernel_spmd
from concourse.masks import make_identity


# ---- problem constants (hardcoded per spec) ----
B, C, T, F = 8, 4, 1000, 1025
N_CORES = 8
SEG = 32
FOFF = 22  # grid phase: f + FOFF = 32*j + u; boundaries at f = 10 (mod 32)
NSEG = (F - 1 + FOFF) // SEG + 1  # 33
CPL = NSEG * SEG  # 1056, c-plane width in staging buffers
GW = NSEG * 128  # packed width: 4224
T_BLOCKS = [(0, 128), (128, 384), (512, 488)]
P = 128

_F32 = mybir.dt.float32
_F16 = mybir.dt.float16


def _build_bands():
    f, interval = 0, 4
    groups = []
    while f < F:
        end = min(f + interval, F)
        groups.append((f, end))
        f = end
        if interval < 32:
            interval += 1
    return groups  # list of (start, end), disjoint, covering [0, F)


def _block_structure():
    """Nonzero (j_out, j_in) block pairs, grouped by j_out (ascending j_in)."""
    bands = _build_bands()
    pairs = set()
    for start, end in bands:
        segs = set(range((start + FOFF) // SEG, (end - 1 + FOFF) // SEG + 1))
        for ji in segs:
            for jo in segs:
                pairs.add((jo, ji))
    jin_lists = [sorted(ji for (jo, ji) in pairs if jo == j) for j in range(NSEG)]
    return bands, jin_lists


def _build_weight_blocks(w_pre, w_post):
    """Host: fold per-band linears and scatter into segment-pair blocks."""
    bands, jin_lists = _block_structure()
    wc = np.einsum(
        "kio,kod->kid", w_pre.astype(np.float64), w_post.astype(np.float64)
    )  # [45, 128, 128], both feature dims indexed by w*4 + c
    blocks = {}
    for k, (start, end) in enumerate(bands):
        fs = np.arange(start, end)
        js = (fs + FOFF) // SEG
        us = (fs + FOFF) % SEG
        for ji in np.unique(js):
            for jo in np.unique(js):
                key = (int(jo), int(ji))
                if key not in blocks:
                    blocks[key] = np.zeros((P, P), dtype=np.float64)
                blk = blocks[key]
                mi = js == ji
                mo = js == jo
                wi = fs[mi] - start
                wo = fs[mo] - start
                for ci in range(C):
                    for co in range(C):
                        blk[np.ix_(ci * SEG + us[mi], co * SEG + us[mo])] = wc[k][
                            np.ix_(wi * C + ci, wo * C + co)
                        ]
    order = [(jo, ji) for jo in range(NSEG) for ji in jin_lists[jo]]
    wall = np.stack([blocks[key] for key in order]).astype(np.float16)
    offs = np.cumsum([0] + [len(jl) for jl in jin_lists])
    return wall, jin_lists, offs


def _bias_field(bands, b_pre, w_post, b_post):
    """bias[c, f]: the constant added to out[., c, ., f]."""
    bc = (
        np.einsum("ko,kod->kd", b_pre.astype(np.float64), w_post.astype(np.float64))
        + b_post.astype(np.float64)
    )
    field = np.zeros((C, F), dtype=np.float64)
    for k, (start, end) in enumerate(bands):
        for c in range(C):
            field[c, start:end] = bc[k, (np.arange(end - start)) * C + c]
    return field.astype(np.float32)


def _t_chunks(t0, tlen):
    out = []
    off = 0
    while off < tlen:
        n = min(P, tlen - off)
        out.append((t0 + off, off, n))
        off += n
    return out


def _build_nc(jin_lists, offs, nblk):
    nc = bacc.Bacc("TRN2", target_bir_lowering=False, debug=False)
    xs = nc.dram_tensor("xs", [C, T, F], _F32, kind="ExternalInput")
    wall = nc.dram_tensor("wall", [nblk, P, P], _F16, kind="ExternalInput")
    ys = nc.dram_tensor("ys", [C, T, F], _F32, kind="ExternalOutput")

    with tile.TileContext(nc) as tc:
        import contextlib

        ctx = contextlib.ExitStack()
        with ctx:
            const_pool = ctx.enter_context(tc.tile_pool(name="const", bufs=1))
            stg_pool = ctx.enter_context(tc.tile_pool(name="stg", bufs=3))
            packed_pool = ctx.enter_context(tc.tile_pool(name="packed", bufs=7))
            ystg_pool = ctx.enter_context(tc.tile_pool(name="ystg", bufs=6))
            at_pool = ctx.enter_context(tc.tile_pool(name="atseg", bufs=8))
            yt_pool = ctx.enter_context(tc.tile_pool(name="ytseg", bufs=8))
            ps_at_pool = ctx.enter_context(
                tc.tile_pool(name="psat", bufs=3, space="PSUM")
            )
            ps_y_pool = ctx.enter_context(
                tc.tile_pool(name="psy", bufs=3, space="PSUM")
            )
            ps_o_pool = ctx.enter_context(
                tc.tile_pool(name="pso", bufs=2, space="PSUM")
            )

            ident = const_pool.tile([P, P], _F16)
            make_identity(nc, ident[:])


            # resident fp16 weights: [128, nblk*128]
            wall_sb = const_pool.tile([P, nblk * P], _F16)
            nc.scalar.dma_start(
                wall_sb[:].rearrange("p (n o) -> p n o", o=P),
                wall.ap().rearrange("n p o -> p n o"),
            )

            def load_and_pack(t0, tlen):
                packed = []
                for tglob, toff, ntc in _t_chunks(t0, tlen):
                    stg = stg_pool.tile([P, C * CPL], _F32, name="stg")
                    for c in range(C):
                        nc.sync.dma_start(
                            stg[0:ntc, c * CPL : c * CPL + F],
                            xs.ap()[c, tglob : tglob + ntc, :],
                        )
                        nc.gpsimd.memset(stg[0:ntc, c * CPL + F : (c + 1) * CPL], 0.0)
                    pk = packed_pool.tile([P, GW], _F16, name="pk")
                    # seg 0 covers f in [-FOFF, SEG-FOFF): zero the pad rows
                    nc.gpsimd.memset(pk[0:ntc, 0:P], 0.0)
                    for c in range(C):
                        # seg 0: f 0..SEG-FOFF-1 at u FOFF..SEG-1
                        nc.vector.tensor_copy(
                            pk[0:ntc, c * SEG + FOFF : (c + 1) * SEG],
                            stg[0:ntc, c * CPL : c * CPL + SEG - FOFF],
                        )
                        # segs 1..NSEG-1: f contiguous from SEG-FOFF
                        src = stg[
                            0:ntc,
                            c * CPL + SEG - FOFF : c * CPL + SEG - FOFF
                            + (NSEG - 1) * SEG,
                        ].rearrange("p (j u) -> p j u", u=SEG)
                        dst = pk[0:ntc, P:].rearrange(
                            "p (j cc u) -> p j cc u", cc=C, u=SEG
                        )[:, :, c, :]
                        nc.vector.tensor_copy(dst, src)
                    packed.append((pk, toff, ntc))
                return packed

            packed_next = load_and_pack(*T_BLOCKS[0])
            for bi, (t0, tlen) in enumerate(T_BLOCKS):
                chunks = _t_chunks(t0, tlen)
                packed = packed_next
                if bi + 1 < len(T_BLOCKS):
                    packed_next = load_and_pack(*T_BLOCKS[bi + 1])

                ystg = {}
                for _tglob, toff, ntc in chunks:
                    ystg[toff] = ystg_pool.tile([P, C * CPL], _F16, name="ystg")

                # ---- per-segment pipeline ----
                at_segs = {}

                def ensure_seg(j, packed=packed, at_segs=at_segs, tlen=tlen):
                    if j in at_segs:
                        return
                    ps = ps_at_pool.tile([P, 512], _F16, name="psat")
                    for pk, toff, ntc in packed:
                        nc.tensor.transpose(
                            ps[:, toff : toff + ntc],
                            pk[0:ntc, j * P : (j + 1) * P],
                            ident[0:ntc, 0:ntc],
                        )
                    seg = at_pool.tile([P, 512], _F16, name="atseg")
                    if j % 2 == 0:
                        nc.scalar.copy(seg[:, 0:tlen], ps[:, 0:tlen])
                    else:
                        nc.vector.tensor_copy(seg[:, 0:tlen], ps[:, 0:tlen])
                    at_segs[j] = seg

                ytiles = {}
                for j_out in range(NSEG):
                    jins = jin_lists[j_out]
                    nw = len(jins)
                    for j in jins:
                        ensure_seg(j)
                    psy = ps_y_pool.tile([P, 512], _F32, name="psy")
                    w0 = offs[j_out]
                    for i, j in enumerate(jins):
                        nc.tensor.matmul(
                            psy[:, 0:tlen],
                            lhsT=wall_sb[:, (w0 + i) * P : (w0 + i + 1) * P],
                            rhs=at_segs[j][:, 0:tlen],
                            start=(i == 0),
                            stop=(i == nw - 1),
                        )
                    yt = yt_pool.tile([P, 512], _F16, name="ytseg")
                    nc.scalar.copy(yt[:, 0:tlen], psy[:, 0:tlen])
                    ytiles[j_out] = yt

                    # ---- flush group of 4 output segments ----
                    last_in_group = (j_out % 4 == 3) or (j_out == NSEG - 1)
                    if not last_in_group:
                        continue
                    g0 = (j_out // 4) * 4
                    gn = j_out - g0 + 1
                    for _tglob, toff, ntc in chunks:
                        pso = ps_o_pool.tile([P, 512], _F16, name="pso")
                        for jj in range(gn):
                            nc.tensor.transpose(
                                pso[0:ntc, jj * P : (jj + 1) * P],
                                ytiles[g0 + jj][:, toff : toff + ntc],
                                ident[:],
                            )
                        ys_t = ystg[toff]
                        ysr = ys_t[0:ntc].rearrange("p (cc x) -> p cc x", cc=C)
                        if g0 == 0:
                            # seg 0: valid u FOFF.. -> f 0..SEG-FOFF-1
                            nc.vector.tensor_copy(
                                ysr[:, :, 0 : SEG - FOFF],
                                pso[0:ntc, 0:P].rearrange(
                                    "p (cc u) -> p cc u", cc=C
                                )[:, :, FOFF:SEG],
                            )
                            src = pso[0:ntc, P : gn * P].rearrange(
                                "p (jj cc u) -> p jj cc u", cc=C, u=SEG
                            )
                            dst = ysr[
                                :, :, SEG - FOFF : SEG - FOFF + (gn - 1) * SEG
                            ].rearrange("p cc (j u) -> p j cc u", u=SEG)
                            nc.vector.tensor_copy(dst, src)
                        elif g0 + gn - 1 == NSEG - 1:
                            uvalid = F - (SEG * (NSEG - 1) - FOFF)
                            f0 = SEG * (NSEG - 1) - FOFF
                            nc.vector.tensor_copy(
                                ysr[:, :, f0 : f0 + uvalid],
                                pso[0:ntc, 0:P].rearrange(
                                    "p (cc u) -> p cc u", cc=C
                                )[:, :, 0:uvalid],
                            )
                        else:
                            src = pso[0:ntc, 0 : gn * P].rearrange(
                                "p (jj cc u) -> p jj cc u", cc=C, u=SEG
                            )
                            f0 = SEG * g0 - FOFF
                            dst = ysr[:, :, f0 : f0 + gn * SEG].rearrange(
                                "p cc (j u) -> p j cc u", u=SEG
                            )
                            nc.vector.tensor_copy(dst, src)
                        stage_bounds = {3: (0, 490), 6: (490, 874)}
                        gidx = g0 // 4
                        if gidx in stage_bounds and gn == 4:
                            lo, hi = stage_bounds[gidx]
                            tglob_c = t0 + toff
                            for c in range(C):
                                nc.gpsimd.dma_start(
                                    ys.ap()[c, tglob_c : tglob_c + ntc, lo:hi],
                                    ys_t[0:ntc, c * CPL + lo : c * CPL + hi],
                                )
                # ---- store the final f-sliver (cast fp16->fp32) ----
                for tglob, toff, ntc in chunks:
                    for c in range(C):
                        nc.gpsimd.dma_start(
                            ys.ap()[c, tglob : tglob + ntc, 874:F],
                            ystg[toff][0:ntc, c * CPL + 874 : c * CPL + F],
                        )
    nc.compile()
    return nc


_CACHE = {}


def kernel(x, w_pre, b_pre, w_post, b_post):
    x = np.asarray(x, dtype=np.float32)
    w_pre = np.asarray(w_pre, dtype=np.float32)
    b_pre = np.asarray(b_pre, dtype=np.float32)
    w_post = np.asarray(w_post, dtype=np.float32)
    b_post = np.asarray(b_post, dtype=np.float32)

    bands, _ = _block_structure()
    wall, jin_lists, offs = _build_weight_blocks(w_pre, w_post)
    nblk = wall.shape[0]

    if "nc" not in _CACHE:
        _CACHE["nc"] = _build_nc(jin_lists, offs, nblk)
    nc = _CACHE["nc"]

    in_maps = [{"xs": x[b], "wall": wall} for b in range(N_CORES)]
    res = run_bass_kernel_spmd(nc, in_maps, core_ids=list(range(N_CORES)))
    out = np.stack([res.results[b]["ys"] for b in range(N_CORES)])

    if np.any(b_pre) or np.any(b_post):
        field = _bias_field(bands, b_pre, w_post, b_post)
        out = out + field[None, :, None, :]
    return out



# revision 3
# speedup vs baseline: 1.9605x; 1.9605x over previous
"""BandSplitLinear Trainium2 kernel (v4: block-diagonal grouped matmul).

Strategy (per core, batch-parallel over 8 cores):
  - No nonlinearity between the two per-band linears -> fold w_pre @ w_post
    into one (w_k*C x w_k*C) matrix per band on the host. Biases are additive
    constants per (c, f) -> applied host-side (zero in practice).
  - In the packed feature order r = f*C + c the folded weight matrix is
    block-diagonal with 45 contiguous square blocks (bands are disjoint,
    contiguous f-ranges). Greedily merge consecutive bands into G groups of
    total width <= 128: the whole computation is G independent matmuls
    y[r0:r0+Wg, :] = Wg_block.T @ x[r0:r0+Wg, :].
  - Host pre-packs x as fp16 [F*C, T] (transposed), so the contraction axis
    is already the partition axis: no on-chip transposes, gathers or
    scatters. Device: load group slab -> matmul (fp16, fp32 PSUM) -> copy
    cast to fp16 -> store. Host unpacks [2, F*C, 500] -> (C, T, F) fp32.
"""

import contextlib

import numpy as np

import concourse.bass as bass
import concourse.tile as tile
from concourse import bacc, mybir
from concourse.bass_utils import run_bass_kernel_spmd


# ---- problem constants (hardcoded per spec) ----
B, C, T, F = 8, 4, 1000, 1025
N_CORES = 8
P = 128
RTOT = F * C  # 4100 packed rows (r = f*C + c)
TC = 500  # matmul free-dim chunk (<= 512 fp32 PSUM bank)
NTC = T // TC  # 2

_F32 = mybir.dt.float32
_F16 = mybir.dt.float16


def _build_bands():
    f, interval = 0, 4
    groups = []
    while f < F:
        end = min(f + interval, F)
        groups.append((f, end))
        f = end
        if interval < 32:
            interval += 1
    return groups  # 45 disjoint (start, end) covering [0, F)


def _make_groups():
    """Greedy consecutive-band grouping, total packed width <= 128."""
    bands = _build_bands()
    sizes = [(e - s) * C for s, e in bands]
    grps, cur, cursum = [], [], 0
    for k, s in enumerate(sizes):
        if cursum + s > P:
            grps.append(cur)
            cur, cursum = [k], s
        else:
            cur.append(k)
            cursum += s
    grps.append(cur)
    meta = [
        (bands[g[0]][0] * C, sum(sizes[k] for k in g), g) for g in grps
    ]  # (r0, wg, band list)
    return meta, sizes, bands


def _build_wall(w_pre, w_post, meta, sizes):
    """Host: fold per-band linears, place group blocks at [0:wg, r0:r0+wg]."""
    wc = np.einsum("kio,kod->kid", w_pre.astype(np.float64), w_post.astype(np.float64))
    wall = np.zeros((P, RTOT), dtype=np.float16)
    for r0, _wg, grp in meta:
        lb = 0
        for k in grp:
            si = sizes[k]
            wall[lb : lb + si, r0 + lb : r0 + lb + si] = wc[k][:si, :si].astype(
                np.float16
            )
            lb += si
    return wall


def _bias_field(bands, b_pre, w_post, b_post):
    """bias[c, f]: the constant added to out[., c, ., f]."""
    bc = (
        np.einsum("ko,kod->kd", b_pre.astype(np.float64), w_post.astype(np.float64))
        + b_post.astype(np.float64)
    )
    field = np.zeros((C, F), dtype=np.float64)
    for k, (start, end) in enumerate(bands):
        for c in range(C):
            field[c, start:end] = bc[k, (np.arange(end - start)) * C + c]
    return field.astype(np.float32)


def _build_nc(meta):
    nc = bacc.Bacc("TRN2", target_bir_lowering=False, debug=False)
    xt = nc.dram_tensor("xt", [RTOT, T], _F16, kind="ExternalInput")
    wall = nc.dram_tensor("wall", [P, RTOT], _F16, kind="ExternalInput")
    ys = nc.dram_tensor("ys", [NTC, RTOT, TC], _F16, kind="ExternalOutput")
    G = len(meta)

    with tile.TileContext(nc) as tc:
        with contextlib.ExitStack() as ctx:
            const_pool = ctx.enter_context(tc.tile_pool(name="const", bufs=1))
            x_pool = ctx.enter_context(tc.tile_pool(name="xg", bufs=6))
            y_pool = ctx.enter_context(tc.tile_pool(name="yg", bufs=8))
            ps_pool = ctx.enter_context(tc.tile_pool(name="ps", bufs=6, space="PSUM"))

            # resident folded weights [128, 4100]; 4-way split load so the
            # first matmul doesn't wait on one long transfer
            wall_sb = const_pool.tile([P, RTOT], _F16)
            queues = [nc.sync, nc.scalar, nc.gpsimd]
            bounds = [0, 1368, 2736, RTOT]
            for i in range(3):
                queues[i].dma_start(
                    wall_sb[:, bounds[i] : bounds[i + 1]],
                    wall.ap()[:, bounds[i] : bounds[i + 1]],
                )

            tiles = {}

            def load(g):
                r0, wg, _ = meta[g]
                t_ = x_pool.tile([P, T], _F16, name="xg")
                nc.sync.dma_start(t_[0:wg, :], xt.ap()[r0 : r0 + wg, :])
                tiles[g] = t_

            PREF = 4
            for g in range(min(PREF, G)):
                load(g)
            for g in range(G):
                r0, wg, _ = meta[g]
                xg = tiles.pop(g)
                for ci in range(NTC):
                    ps = ps_pool.tile([P, TC], _F32, name="ps")
                    nc.tensor.matmul(
                        ps[0:wg, :],
                        lhsT=wall_sb[0:wg, r0 : r0 + wg],
                        rhs=xg[0:wg, ci * TC : (ci + 1) * TC],
                        start=True,
                        stop=True,
                    )
                    yg = y_pool.tile([P, TC], _F16, name="yg")
                    if ci % 2 == 0:
                        nc.scalar.copy(yg[0:wg, :], ps[0:wg, :])
                    else:
                        nc.vector.tensor_copy(yg[0:wg, :], ps[0:wg, :])
                    nc.gpsimd.dma_start(ys.ap()[ci, r0 : r0 + wg, :], yg[0:wg, :])
                if g + PREF < G:
                    load(g + PREF)
    nc.compile()
    return nc


_CACHE = {}


def _prepare(x, w_pre, w_post):
    """Returns (nc, in_maps) ready for run_bass_kernel_spmd."""
    meta, sizes, _bands = _make_groups()
    wall = _build_wall(w_pre, w_post, meta, sizes)
    if "nc" not in _CACHE:
        _CACHE["nc"] = _build_nc(meta)
    xt_all = np.ascontiguousarray(
        x.transpose(0, 3, 1, 2).reshape(B, RTOT, T), dtype=np.float16
    )
    in_maps = [{"xt": xt_all[b], "wall": wall} for b in range(N_CORES)]
    return _CACHE["nc"], in_maps


def kernel(x, w_pre, b_pre, w_post, b_post):
    x = np.asarray(x, dtype=np.float32)
    w_pre = np.asarray(w_pre, dtype=np.float32)
    b_pre = np.asarray(b_pre, dtype=np.float32)
    w_post = np.asarray(w_post, dtype=np.float32)
    b_post = np.asarray(b_post, dtype=np.float32)

    nc, in_maps = _prepare(x, w_pre, w_post)
    res = run_bass_kernel_spmd(nc, in_maps, core_ids=list(range(N_CORES)))
    ys_all = np.stack([res.results[b]["ys"] for b in range(N_CORES)])

    # [B, NTC, F*C, TC] -> [B, C, T, F]
    out = (
        ys_all.reshape(B, NTC, F, C, TC)
        .transpose(0, 3, 1, 4, 2)
        .reshape(B, C, T, F)
        .astype(np.float32)
    )

    if np.any(b_pre) or np.any(b_post):
        field = _bias_field(_build_bands(), b_pre, w_post, b_post)
        out = out + field[None, :, None, :]
    return out


# revision 6
# speedup vs baseline: 2.9740x; 1.5170x over previous
"""BandSplitLinear Trainium2 kernel (v5: bin-packed 128-row stripes, big DMAs).

Strategy (per core, batch-parallel over 8 cores):
  - No nonlinearity between the two per-band linears -> fold w_pre @ w_post
    into one (w_k*C x w_k*C) matrix per band on the host. Biases are additive
    constants per (c, f) -> applied host-side (zero in practice).
  - In packed feature order r = f*C + c the folded weight matrix is
    block-diagonal with 45 square blocks. Bin-pack the 45 bands into G=33
    bins of total width <= 128 (optimal: ceil(4100/128)); the host lays x
    out as fp16 [G, 128, T] stripes (pad rows zero-weighted), so the whole
    model is G uniform 128x128 matmuls with the contraction axis already on
    partitions: no transposes, gathers or scatters on device.
  - DMA instruction count is the bottleneck (~630ns serialized descriptor
    generation per dma_start), so x loads / y stores are batched into
    multi-stripe transfers; weights are one DMA. fp16 in/out halves HBM
    traffic; fp32 accumulate in PSUM.
"""

import contextlib

import numpy as np

import concourse.bass as bass
import concourse.tile as tile
from concourse import bacc, mybir
from concourse.bass_utils import run_bass_kernel_spmd


# ---- problem constants (hardcoded per spec) ----
B, C, T, F = 8, 4, 1000, 1025
N_CORES = 8
P = 128
RTOT = F * C  # 4100 dense packed rows (r = f*C + c)
TC = 500  # matmul free-dim chunk (<= 512 fp32 PSUM bank)
NTC = T // TC  # 2
SPLIT = 3  # stripes per DMA piece

_F32 = mybir.dt.float32
_F16 = mybir.dt.float16


def _build_bands():
    f, interval = 0, 4
    groups = []
    while f < F:
        end = min(f + interval, F)
        groups.append((f, end))
        f = end
        if interval < 32:
            interval += 1
    return groups  # 45 disjoint (start, end) covering [0, F)


def _make_bins():
    """First-fit-decreasing bin packing of band widths into 128-row bins."""
    bands = _build_bands()
    sizes = [(e - s) * C for s, e in bands]
    order = sorted(range(len(sizes)), key=lambda k: -sizes[k])
    bins, fill = [], []
    for k in order:
        for i in range(len(bins)):
            if fill[i] + sizes[k] <= P:
                bins[i].append(k)
                fill[i] += sizes[k]
                break
        else:
            bins.append([k])
            fill.append(sizes[k])
    for b in bins:
        b.sort()
    bins.sort(key=lambda b: b[0])
    return bands, sizes, bins


def _layout():
    """perm: padded row -> dense row (or -1); pos: dense row -> padded row."""
    bands, sizes, bins = _make_bins()
    G = len(bins)
    perm = np.zeros(G * P, dtype=np.int64)  # pad rows point at 0 (zero weight)
    pos = np.empty(RTOT, dtype=np.int64)
    for g, bn in enumerate(bins):
        lb = 0
        for k in bn:
            si = sizes[k]
            dense0 = bands[k][0] * C
            perm[g * P + lb : g * P + lb + si] = np.arange(dense0, dense0 + si)
            pos[dense0 : dense0 + si] = np.arange(g * P + lb, g * P + lb + si)
            lb += si
    return bands, sizes, bins, G, perm, pos


def _build_wall(w_pre, w_post, sizes, bins, bands, G):
    """Host: fold per-band linears, place blocks diagonally inside each bin."""
    wc = np.einsum("kio,kod->kid", w_pre.astype(np.float64), w_post.astype(np.float64))
    wall = np.zeros((P, G * P), dtype=np.float16)
    for g, bn in enumerate(bins):
        lb = 0
        for k in bn:
            si = sizes[k]
            wall[lb : lb + si, g * P + lb : g * P + lb + si] = wc[k][:si, :si].astype(
                np.float16
            )
            lb += si
    return wall


def _bias_field(bands, b_pre, w_post, b_post):
    """bias[c, f]: the constant added to out[., c, ., f]."""
    bc = (
        np.einsum("ko,kod->kd", b_pre.astype(np.float64), w_post.astype(np.float64))
        + b_post.astype(np.float64)
    )
    field = np.zeros((C, F), dtype=np.float64)
    for k, (start, end) in enumerate(bands):
        for c in range(C):
            field[c, start:end] = bc[k, (np.arange(end - start)) * C + c]
    return field.astype(np.float32)


def _pieces(G):
    out = []
    s = 0
    while s * SPLIT < G:
        out.append((s * SPLIT, min((s + 1) * SPLIT, G)))
        s += 1
    return out  # [(g0, g1)) stripe ranges per DMA piece


def _build_nc(G):
    nc = bacc.Bacc("TRN2", target_bir_lowering=False, debug=False)
    xt = nc.dram_tensor("xt", [G, P, T], _F16, kind="ExternalInput")
    wall = nc.dram_tensor("wall", [P, G * P], _F16, kind="ExternalInput")
    ys = nc.dram_tensor("ys", [G, P, T], _F16, kind="ExternalOutput")
    pieces = _pieces(G)

    with tile.TileContext(nc) as tc:
        with contextlib.ExitStack() as ctx:
            const_pool = ctx.enter_context(tc.tile_pool(name="const", bufs=1))
            x_pool = ctx.enter_context(tc.tile_pool(name="xp", bufs=len(pieces)))
            y_pool = ctx.enter_context(tc.tile_pool(name="yp", bufs=len(pieces)))
            ps_pool = ctx.enter_context(tc.tile_pool(name="ps", bufs=8, space="PSUM"))

            wall_sb = const_pool.tile([P, G * P], _F16)
            nc.scalar.dma_start(wall_sb[:], wall.ap())

            xp, yp = [], []
            for g0, g1 in pieces:
                n = g1 - g0
                t_ = x_pool.tile([P, n * T], _F16, name="xp")
                nc.sync.dma_start(
                    t_[:].rearrange("p (i t) -> p i t", t=T),
                    xt.ap()[g0:g1].rearrange("i p t -> p i t"),
                )
                xp.append(t_)
                yp.append(y_pool.tile([P, n * T], _F16, name="yp"))

            copy_engines = [
                lambda d, s_: nc.vector.tensor_copy(d, s_),
                lambda d, s_: nc.scalar.copy(d, s_),
            ]
            nco = 0
            for s, (g0, g1) in enumerate(pieces):
                for g in range(g0, g1):
                    o = g - g0
                    for ci in range(NTC):
                        ps = ps_pool.tile([P, TC], _F32, name="ps")
                        nc.tensor.matmul(
                            ps[:],
                            lhsT=wall_sb[:, g * P : (g + 1) * P],
                            rhs=xp[s][:, o * T + ci * TC : o * T + (ci + 1) * TC],
                            start=True,
                            stop=True,
                        )
                        copy_engines[nco % 2](
                            yp[s][:, o * T + ci * TC : o * T + (ci + 1) * TC], ps[:]
                        )
                        nco += 1
                nc.sync.dma_start(
                    ys.ap()[g0:g1].rearrange("i p t -> p i t"),
                    yp[s][:].rearrange("p (i t) -> p i t", t=T),
                )
    nc.compile()
    return nc


_CACHE = {}


def _prepare(x, w_pre, w_post):
    """Returns (nc, in_maps) ready for run_bass_kernel_spmd."""
    bands, sizes, bins, G, perm, _pos = _layout()
    wall = _build_wall(w_pre, w_post, sizes, bins, bands, G)
    if "nc" not in _CACHE:
        _CACHE["nc"] = _build_nc(G)
    xt16 = np.ascontiguousarray(
        x.transpose(0, 3, 1, 2).reshape(B, RTOT, T), dtype=np.float16
    )
    xt_pad = xt16[:, perm, :].reshape(B, G, P, T)
    in_maps = [{"xt": xt_pad[b], "wall": wall} for b in range(N_CORES)]
    return _CACHE["nc"], in_maps


def kernel(x, w_pre, b_pre, w_post, b_post):
    x = np.asarray(x, dtype=np.float32)
    w_pre = np.asarray(w_pre, dtype=np.float32)
    b_pre = np.asarray(b_pre, dtype=np.float32)
    w_post = np.asarray(w_post, dtype=np.float32)
    b_post = np.asarray(b_post, dtype=np.float32)

    bands, _sizes, _bins, G, _perm, pos = _layout()
    nc, in_maps = _prepare(x, w_pre, w_post)
    res = run_bass_kernel_spmd(nc, in_maps, core_ids=list(range(N_CORES)))
    ys_all = np.stack([res.results[b]["ys"] for b in range(N_CORES)])

    # [B, G*P, T] -> dense rows -> [B, C, T, F]
    yt = ys_all.reshape(B, G * P, T)[:, pos, :]
    out = (
        yt.reshape(B, F, C, T).transpose(0, 2, 3, 1).astype(np.float32)
    )

    if np.any(b_pre) or np.any(b_post):
        field = _bias_field(bands, b_pre, w_post, b_post)
        out = out + field[None, :, None, :]
    return out


# revision 8
# speedup vs baseline: 2.9815x; 1.0025x over previous
"""BandSplitLinear Trainium2 kernel (v5: bin-packed 128-row stripes, big DMAs).

Strategy (per core, batch-parallel over 8 cores):
  - No nonlinearity between the two per-band linears -> fold w_pre @ w_post
    into one (w_k*C x w_k*C) matrix per band on the host. Biases are additive
    constants per (c, f) -> applied host-side (zero in practice).
  - In packed feature order r = f*C + c the folded weight matrix is
    block-diagonal with 45 square blocks. Bin-pack the 45 bands into G=33
    bins of total width <= 128 (optimal: ceil(4100/128)); the host lays x
    out as fp16 [G, 128, T] stripes (pad rows zero-weighted), so the whole
    model is G uniform 128x128 matmuls with the contraction axis already on
    partitions: no transposes, gathers or scatters on device.
  - DMA instruction count is the bottleneck (~630ns serialized descriptor
    generation per dma_start), so x loads / y stores are batched into
    multi-stripe transfers; weights are one DMA. fp16 in/out halves HBM
    traffic; fp32 accumulate in PSUM.
"""

import contextlib

import numpy as np

import concourse.bass as bass
import concourse.tile as tile
from concourse import bacc, mybir
from concourse.bass_utils import run_bass_kernel_spmd


# ---- problem constants (hardcoded per spec) ----
B, C, T, F = 8, 4, 1000, 1025
N_CORES = 8
P = 128
RTOT = F * C  # 4100 dense packed rows (r = f*C + c)
TC = 500  # matmul free-dim chunk (<= 512 fp32 PSUM bank)
NTC = T // TC  # 2
SPLIT = 3  # stripes per DMA piece

_F32 = mybir.dt.float32
_F16 = mybir.dt.float16


def _build_bands():
    f, interval = 0, 4
    groups = []
    while f < F:
        end = min(f + interval, F)
        groups.append((f, end))
        f = end
        if interval < 32:
            interval += 1
    return groups  # 45 disjoint (start, end) covering [0, F)


def _make_bins():
    """First-fit-decreasing bin packing of band widths into 128-row bins."""
    bands = _build_bands()
    sizes = [(e - s) * C for s, e in bands]
    order = sorted(range(len(sizes)), key=lambda k: -sizes[k])
    bins, fill = [], []
    for k in order:
        for i in range(len(bins)):
            if fill[i] + sizes[k] <= P:
                bins[i].append(k)
                fill[i] += sizes[k]
                break
        else:
            bins.append([k])
            fill.append(sizes[k])
    for b in bins:
        b.sort()
    bins.sort(key=lambda b: b[0])
    return bands, sizes, bins


def _layout():
    """perm: padded row -> dense row (or -1); pos: dense row -> padded row."""
    bands, sizes, bins = _make_bins()
    G = len(bins)
    perm = np.zeros(G * P, dtype=np.int64)  # pad rows point at 0 (zero weight)
    pos = np.empty(RTOT, dtype=np.int64)
    for g, bn in enumerate(bins):
        lb = 0
        for k in bn:
            si = sizes[k]
            dense0 = bands[k][0] * C
            perm[g * P + lb : g * P + lb + si] = np.arange(dense0, dense0 + si)
            pos[dense0 : dense0 + si] = np.arange(g * P + lb, g * P + lb + si)
            lb += si
    return bands, sizes, bins, G, perm, pos


def _build_wall(w_pre, w_post, sizes, bins, bands, G):
    """Host: fold per-band linears, place blocks diagonally inside each bin."""
    wc = np.einsum("kio,kod->kid", w_pre.astype(np.float64), w_post.astype(np.float64))
    wall = np.zeros((P, G * P), dtype=np.float16)
    for g, bn in enumerate(bins):
        lb = 0
        for k in bn:
            si = sizes[k]
            wall[lb : lb + si, g * P + lb : g * P + lb + si] = wc[k][:si, :si].astype(
                np.float16
            )
            lb += si
    return wall


def _bias_field(bands, b_pre, w_post, b_post):
    """bias[c, f]: the constant added to out[., c, ., f]."""
    bc = (
        np.einsum("ko,kod->kd", b_pre.astype(np.float64), w_post.astype(np.float64))
        + b_post.astype(np.float64)
    )
    field = np.zeros((C, F), dtype=np.float64)
    for k, (start, end) in enumerate(bands):
        for c in range(C):
            field[c, start:end] = bc[k, (np.arange(end - start)) * C + c]
    return field.astype(np.float32)


def _pieces(G):
    out = []
    s = 0
    while s * SPLIT < G:
        out.append((s * SPLIT, min((s + 1) * SPLIT, G)))
        s += 1
    return out  # [(g0, g1)) stripe ranges per DMA piece


def _build_nc(G):
    nc = bacc.Bacc("TRN2", target_bir_lowering=False, debug=False)
    xt = nc.dram_tensor("xt", [G, P, T], _F16, kind="ExternalInput")
    wall = nc.dram_tensor("wall", [P, G * P], _F16, kind="ExternalInput")
    ys = nc.dram_tensor("ys", [G, P, T], _F16, kind="ExternalOutput")
    pieces = _pieces(G)

    with tile.TileContext(nc) as tc:
        with contextlib.ExitStack() as ctx:
            const_pool = ctx.enter_context(tc.tile_pool(name="const", bufs=1))
            x_pool = ctx.enter_context(tc.tile_pool(name="xp", bufs=len(pieces)))
            y_pool = ctx.enter_context(tc.tile_pool(name="yp", bufs=len(pieces)))
            ps_pool = ctx.enter_context(tc.tile_pool(name="ps", bufs=8, space="PSUM"))

            wall_sb = const_pool.tile([P, G * P], _F16)
            nc.gpsimd.dma_start(wall_sb[:], wall.ap())

            xp, yp = [], []
            for pi, (g0, g1) in enumerate(pieces):
                n = g1 - g0
                t_ = x_pool.tile([P, n * T], _F16, name="xp")
                lq = nc.sync if pi % 2 == 0 else nc.scalar
                lq.dma_start(
                    t_[:].rearrange("p (i t) -> p i t", t=T),
                    xt.ap()[g0:g1].rearrange("i p t -> p i t"),
                )
                xp.append(t_)
                yp.append(y_pool.tile([P, n * T], _F16, name="yp"))

            copy_engines = [
                lambda d, s_: nc.vector.tensor_copy(d, s_),
                lambda d, s_: nc.scalar.copy(d, s_),
            ]
            nco = 0
            for s, (g0, g1) in enumerate(pieces):
                for g in range(g0, g1):
                    o = g - g0
                    for ci in range(NTC):
                        ps = ps_pool.tile([P, TC], _F32, name="ps")
                        nc.tensor.matmul(
                            ps[:],
                            lhsT=wall_sb[:, g * P : (g + 1) * P],
                            rhs=xp[s][:, o * T + ci * TC : o * T + (ci + 1) * TC],
                            start=True,
                            stop=True,
                        )
                        copy_engines[nco % 2](
                            yp[s][:, o * T + ci * TC : o * T + (ci + 1) * TC], ps[:]
                        )
                        nco += 1
                nc.gpsimd.dma_start(
                    ys.ap()[g0:g1].rearrange("i p t -> p i t"),
                    yp[s][:].rearrange("p (i t) -> p i t", t=T),
                )
    nc.compile()
    return nc


_CACHE = {}


def _prepare(x, w_pre, w_post):
    """Returns (nc, in_maps) ready for run_bass_kernel_spmd."""
    bands, sizes, bins, G, perm, _pos = _layout()
    wall = _build_wall(w_pre, w_post, sizes, bins, bands, G)
    if "nc" not in _CACHE:
        _CACHE["nc"] = _build_nc(G)
    xt16 = np.ascontiguousarray(
        x.transpose(0, 3, 1, 2).reshape(B, RTOT, T), dtype=np.float16
    )
    xt_pad = xt16[:, perm, :].reshape(B, G, P, T)
    in_maps = [{"xt": xt_pad[b], "wall": wall} for b in range(N_CORES)]
    return _CACHE["nc"], in_maps


def kernel(x, w_pre, b_pre, w_post, b_post):
    x = np.asarray(x, dtype=np.float32)
    w_pre = np.asarray(w_pre, dtype=np.float32)
    b_pre = np.asarray(b_pre, dtype=np.float32)
    w_post = np.asarray(w_post, dtype=np.float32)
    b_post = np.asarray(b_post, dtype=np.float32)

    bands, _sizes, _bins, G, _perm, pos = _layout()
    nc, in_maps = _prepare(x, w_pre, w_post)
    res = run_bass_kernel_spmd(nc, in_maps, core_ids=list(range(N_CORES)))
    ys_all = np.stack([res.results[b]["ys"] for b in range(N_CORES)])

    # [B, G*P, T] -> dense rows -> [B, C, T, F]
    yt = ys_all.reshape(B, G * P, T)[:, pos, :]
    out = (
        yt.reshape(B, F, C, T).transpose(0, 2, 3, 1).astype(np.float32)
    )

    if np.any(b_pre) or np.any(b_post):
        field = _bias_field(bands, b_pre, w_post, b_post)
        out = out + field[None, :, None, :]
    return out
